# revision 8
# baseline (speedup 1.0000x reference)
"""Cost-volume builder (correlation layer) for Trainium2, 8-core SPMD.

out[b, d, h, w] = (1/sqrt(C)) * sum_c feat1[b,c,h,w] * feat2[b,c,h+dy,w+dx]
for d = (dy+4)*9 + (dx+4), dy,dx in [-4,4]. B,C,H,W = 4,128,192,256.

Sharding: 8 cores = 4 batches x 2 H-halves (96 rows each, feat2 halo +-4).

Per-core algorithm, pipelined over three 32-row thirds:
  f1 resident: [C, 96 x W] fp16. All input loads ride the sync HWDGE
    queue in just-in-time order (one queue saturates HBM; two split it).
  Pass 1 (PE), flat over 13 f2 slabs g: per g x w-tile (8 outputs wide,
    16-wide feat2 window): matmul lhsT=F2win[C,16x8=128] vs
    rhs=F1[C,16 h-rows x 8 w] -> band[(we,j),(h,w)] in PSUM. g=0/12 are
    half-width; g=4/8 straddle two thirds (full-width MM, split copy).
    8 matmuls share a 2-bank PSUM tile -> 1-2 stage copies.
  Stage (DVE/ACT rotate): PSUM->SBUF fp16 cast; col = cls*160 + slot*32
    + w0, cls = h_off*8 + wl, slot = within-third r-block index. Two
    stage buffers -> third Ti+1's pass 1 overlaps third Ti's pass 2.
  Pass 2 (PE), transposed: stationary = stage class block [C',128]
    (fast weight load), moving = one-hot fp8->fp16 selection matrix
    Sel[C',81] (DMAed fp8, SWDGE casts in flight). Two accumulated
    matmuls (class t slots 0..3, class t-8 slots 1..4) per (t,wl) ->
    psum[pixel 128, wl slot 128 + d]. One copy -> [128, 8*81] fp16
    phase tile, one 128-partition store per phase (scalar/gpsimd DMA
    queues; sync stays on loads). Host restores [D, OH, W] fp32 from
    the permuted fp16 layout [pixel(a,w0)][Ti][t][wl][d].
"""

import math

import numpy as np

B, C, H, W = 4, 128, 192, 256
D = 81
NCORES = 8
OH = H // 2            # 96 output rows per core
NT = 3                 # thirds of 32 rows
NSLAB = 13             # f2 slabs per core
NKS = 5                # stage col slots per class (5 * 32 w-tiles)
T = 8
F2W = W + 8            # 264, zero-padded W
SCALE = 1.0 / math.sqrt(C)
PHC = 8 * D            # 648 out cols per phase

# f2 slab -> (tile index, slot) for load chunks [(0,),(1,2),...,(11,12)]
_F2CHUNKS = [(0,), (1, 2), (3, 4), (5, 6), (7, 8), (9, 10), (11, 12)]
_SLAB2TILE = {}
for _i, _ch in enumerate(_F2CHUNKS):
    for _s, _g in enumerate(_ch):
        _SLAB2TILE[_g] = (_i, _s)

# f1 row chunks; chunk i must land before slab chunk i gates pass-1
_F1CHUNKS = [(0, 8), (8, 24), (24, 40), (40, 56), (56, 72), (72, 88), (88, 96)]


def _build_sel():
    """[128, 128*81] fp8 one-hot selection matrices, cls = h_off*8 + wl.
    Weight-column order of pass-1 lhsT is (kappa, j): row = kappa*8 + j."""
    import ml_dtypes

    sel = np.zeros((128, 128 * 81), np.float32)
    for h_off in range(16):
        for wl in range(8):
            c0 = (h_off * 8 + wl) * 81
            for j in range(8):
                dy = j + 4 - h_off
                if -4 <= dy <= 4:
                    for dxh in range(9):  # dxh = dx + 4
                        row = (wl + dxh) * 8 + j
                        sel[row, c0 + (dy + 4) * 9 + dxh] = 1.0
    return sel.astype(ml_dtypes.float8_e4m3)


def _emit(tc, f1, f2, selt, out):
    """Emit the Tile program. f1:[C,96*W] f16, f2:[C,13*F2W*8] f16,
    selt:[C,128*81] fp8e4, out:[128, 24*648] f16 (DRAM APs)."""
    import concourse.bass as bass
    import concourse.mybir as mybir

    dt = mybir.dt
    nc = tc.nc
    MS = bass.MemorySpace

    # GPSIMD cannot access PSUM, so PSUM->SBUF copies rotate DVE/ACT only.
    copy_engines = [nc.vector.tensor_copy, nc.scalar.copy]
    eng = [0]

    def copy(dst, src):
        copy_engines[eng[0]](dst, src)
        eng[0] = (eng[0] + 1) % 2

    # store queues: ACT HWDGE and SWDGE (sync is dedicated to loads)
    st_engines = [nc.scalar, nc.gpsimd]
    st_idx = [0]

    def store(dst, src):
        st_engines[st_idx[0]].dma_start(dst, src)
        st_idx[0] = (st_idx[0] + 1) % 2

    with (
        tc.tile_pool(name="const", bufs=1) as cpool,
        tc.tile_pool(name="f1p", bufs=1) as f1p,
        tc.tile_pool(name="f2p", bufs=4) as f2p,
        tc.tile_pool(name="stgp", bufs=2) as stgp,
        tc.tile_pool(name="outp", bufs=4) as outp,
        tc.tile_pool(name="ps", bufs=4, space=MS.PSUM) as psp,
    ):
        # HAM warm-up: ~4us of throwaway matmuls so the PE clock gate is
        # released (1.2 -> 2.4 GHz) before the first real matmul.
        warm = cpool.tile([128, 512], dt.float16, tag="warm")
        nc.vector.memset(warm[:, :], 0.0)
        for _ in range(5):
            wpt = psp.tile([128, 1024], dt.float32, tag="ps1")
            for v in range(2):
                nc.tensor.matmul(
                    wpt[:, v * 512 : v * 512 + 512],
                    warm[:, 0:128],
                    warm[:, :],
                    start=True,
                    stop=True,
                )

        # sel: fp8 in DRAM (plain fast DMA on the idle SWDGE queue), then
        # cast fp8 -> fp16 on DVE while it is otherwise idle. A casting
        # DMA would hog all 16 SDMA engines at ~1/8 rate and starve the
        # critical f1/f2 loads.
        sel8 = cpool.tile([128, 128 * 81], dt.float8e4, tag="sel8")
        nc.gpsimd.dma_start(sel8[:, :], selt[:, :])
        selb = cpool.tile([128, 128 * 81], dt.float16)
        HS = 64 * 81
        nc.vector.tensor_copy(selb[:, 0:HS], sel8[:, 0:HS])
        nc.vector.tensor_copy(selb[:, HS : 128 * 81], sel8[:, HS : 128 * 81])

        # resident f1 rows 0..95 (global h), loaded in chunks below
        f1h = f1p.tile([128, OH * W], dt.float16, tag="f1h")
        f1v = f1h[:, :].rearrange("p (h x) -> p h x", h=OH)

        f2tiles = [None] * len(_F2CHUNKS)

        def load_f2(i):
            ch = _F2CHUNKS[i]
            t2 = f2p.tile([128, 2 * F2W * 8], dt.float16, tag="f2s")
            nc.sync.dma_start(
                t2[:, 0 : len(ch) * F2W * 8],
                f2[:, ch[0] * F2W * 8 : (ch[-1] + 1) * F2W * 8],
            )
            f2tiles[i] = t2

        def load_f1(i):
            lo, hi = _F1CHUNKS[i]
            nc.sync.dma_start(f1h[:, lo * W : hi * W], f1[:, lo * W : hi * W])

        def slab(g):
            ti, sl = _SLAB2TILE[g]
            return f2tiles[ti][:, sl * F2W * 8 : (sl + 1) * F2W * 8]

        # JIT load order: f1 chunk i lands before f2 chunk i gates compute
        load_f2(0)
        load_f1(0)
        load_f1(1)
        load_f2(1)
        load_f1(2)
        load_f2(2)
        load_f1(3)
        load_f2(3)
        load_f1(4)
        load_f2(4)
        load_f1(5)
        load_f2(5)
        load_f1(6)
        load_f2(6)

        # stage tiles, allocated on demand (bufs=2 pipelines thirds)
        stv = [None] * NT

        def alloc_stage(ti):
            stg = stgp.tile([128, 128 * NKS * 32], dt.float16, tag="stg")
            stv[ti] = stg[:, :].rearrange("p (c t) -> p c t", c=128)

        def pass2(ti):
            for t in range(8, 16):
                ot = outp.tile([128, PHC], dt.float16, tag="outt")
                p2 = psp.tile([128, 1024], dt.float32, tag="ps1")
                for wl in range(8):
                    clsA = t * 8 + wl
                    clsB = (t - 8) * 8 + wl
                    dst2 = p2[:, 128 * wl : 128 * wl + 81]
                    nc.tensor.matmul(
                        dst2,
                        stv[ti][:, clsA, 0:128],    # slots k=0..3
                        selb[:, 81 * clsA : 81 * clsA + 81],
                        start=True,
                        stop=False,
                    )
                    nc.tensor.matmul(
                        dst2,
                        stv[ti][:, clsB, 32:160],   # slots k=1..4
                        selb[:, 81 * clsB : 81 * clsB + 81],
                        start=False,
                        stop=True,
                    )
                src = p2[:, :].rearrange("p (s x) -> p s x", s=8)[:, :, 0:D]
                dst = ot[:, :].rearrange("p (s d) -> p s d", s=8)
                copy(dst, src)
                store(
                    out[:, (ti * 8 + t - 8) * PHC : (ti * 8 + t - 7) * PHC],
                    ot[:, :],
                )

        # ---- pass 1: flat over 13 slabs; pass 2 per completed third ----
        for g in range(NSLAB):
            if g % 4 == 0 and g // 4 < NT:
                alloc_stage(g // 4)
            f2s = slab(g)
            rlo = max(0, 8 * g - 8)
            mw = 8 if g in (0, NSLAB - 1) else 16
            nmv = mw * 8
            for gq in range(4):  # groups of 8 w-tiles per 2-bank PSUM tile
                pt = psp.tile([128, 1024], dt.float32, tag="ps1")
                for u in range(8):
                    w0 = gq * 8 + u
                    lhsT = f2s[:, 64 * w0 : 64 * w0 + 128]     # [128,128]
                    rhs = f1v[:, rlo : rlo + mw, 8 * w0 : 8 * w0 + T]
                    nc.tensor.matmul(
                        pt[:, u * nmv : (u + 1) * nmv],
                        lhsT,
                        rhs,
                        start=True,
                        stop=True,
                    )
                t0 = (g % 4) * 32 + gq * 8
                src = pt[:, 0 : 8 * nmv].rearrange("p (u c) -> p c u", u=8)
                if g == 0:
                    # h_off 8..15 -> classes 64.. of T0 slot 0
                    copy(stv[0][:, 64:128, gq * 8 : gq * 8 + 8], src)
                elif g == NSLAB - 1:
                    # h_off 0..7 -> classes 0..63 of T2 slot 4
                    copy(stv[2][:, 0:64, 128 + gq * 8 : 136 + gq * 8], src)
                elif g % 4 == 0:
                    # straddles thirds: lower classes -> T(g/4-1) slot 4,
                    # upper classes -> T(g/4) slot 0
                    copy(
                        stv[g // 4 - 1][:, 0:64, 128 + gq * 8 : 136 + gq * 8],
                        src[:, 0:64, :],
                    )
                    copy(
                        stv[g // 4][:, 64:128, gq * 8 : gq * 8 + 8],
                        src[:, 64:128, :],
                    )
                else:
                    copy(stv[g // 4][:, :, t0 : t0 + 8], src)
            if g in (4, 8, 12):
                pass2(g // 4 - 1)


def _build_nc():
    import concourse.mybir as mybir
    import concourse.tile as tile
    from concourse import bacc

    dt = mybir.dt
    nc = bacc.Bacc("TRN2", target_bir_lowering=False, debug=False)
    f1 = nc.dram_tensor("f1", [C, OH * W], dt.float16, kind="ExternalInput")
    f2 = nc.dram_tensor(
        "f2", [C, NSLAB * F2W * 8], dt.float16, kind="ExternalInput"
    )
    selt = nc.dram_tensor("sel", [C, 128 * 81], dt.float8e4, kind="ExternalInput")
    out = nc.dram_tensor("out", [128, 24 * PHC], dt.float16, kind="ExternalOutput")
    with tile.TileContext(nc) as tc:
        _emit(tc, f1[:, :], f2[:, :], selt[:, :], out[:, :])
    nc.finalize()
    return nc


def _shard_inputs(feat1, feat2):
    sel = _build_sel()
    in_maps = []
    for core in range(NCORES):
        b, half = core // 2, core % 2
        h0 = half * OH
        f1s = (feat1[b, :, h0 : h0 + OH, :] * SCALE).astype(np.float16)
        f2pad = np.zeros((C, OH + 8, F2W), np.float16)
        lo, hi = h0 - 4, h0 + OH + 4
        slo, shi = max(lo, 0), min(hi, H)
        f2pad[:, slo - lo : shi - lo, 4 : 4 + W] = feat2[b, :, slo:shi, :].astype(
            np.float16
        )
        # slab g -> f2pad rows [8g, 8g+8) (= global rows 8g-4..8g+4),
        # transposed to [C, w, r] so each (16 w x 8 r) weight window is
        # contiguous.
        slabs = np.zeros((C, NSLAB, F2W, 8), np.float16)
        for g in range(NSLAB):
            slabs[:, g] = f2pad[:, 8 * g : 8 * g + 8, :].transpose(0, 2, 1)
        in_maps.append(
            {
                "f1": np.ascontiguousarray(f1s.reshape(C, OH * W)),
                "f2": np.ascontiguousarray(slabs.reshape(C, -1)),
                "sel": sel,
            }
        )
    return in_maps


def _unshard_out(arr):
    """[128, 24*648] device layout [p=(a,w0)][Ti][tt][wl][d] -> [D, OH, W]."""
    return (
        arr.reshape(4, 32, NT, 8, 8, D)
        .transpose(5, 2, 0, 3, 1, 4)
        .reshape(D, OH, W)
    )


def kernel(feat1, feat2):
    feat1 = np.asarray(feat1, dtype=np.float32)
    feat2 = np.asarray(feat2, dtype=np.float32)
    from concourse.bass_utils import run_bass_kernel_spmd

    nc = _build_nc()
    in_maps = _shard_inputs(feat1, feat2)
    res = run_bass_kernel_spmd(nc, in_maps, list(range(NCORES)))
    full = np.zeros((B, D, H, W), np.float32)
    for core in range(NCORES):
        b, half = core // 2, core % 2
        full[b, :, half * OH : (half + 1) * OH, :] = _unshard_out(
            res.results[core]["out"].astype(np.float32)
        )
    return full
